# revision 3
# baseline (speedup 1.0000x reference)
"""ContrastiveHardestNegativeLoss on 8 Trainium2 NeuronCores (Bass/Tile).

Strategy (per sharding hint): shard the positive-pair (row) dimension of the
P x M distance matrices across the 8 cores. Each core receives:
  - its slice of the gathered pos features, transposed + augmented:
      lhs[d, i] = posF[i, d] for d < 32,  lhs[32, i] = 1.0
  - the full gathered sub features, transposed, scaled and augmented:
      rhs[d, c] = -2 * subF[c, d] for d < 32,  rhs[32, c] = |subF[c]|^2
  so a single PE matmul produces q[i, c] = |subF[c]|^2 - 2 <posF[i], subF[c]>,
  i.e. the squared distance minus the (row-constant) |posF[i]|^2 term, which is
  added back after the column-min reduction (min is invariant to it).

Device per core: 2 matrices x 16 row-tiles x 8 col-chunks of matmul (K=33,
N=512) into PSUM, VectorE min-reduce per chunk, then a tiny epilogue:
dist = sqrt(max(pn + min_q, 0) + 1e-7), n = relu(1.4 - dist)^2, summed across
rows on the PE (ones-matmul). The positive loss relu(|p0-p1|^2 - 0.1) is
computed from the same augmented operands. Output: 3 partial sums per core.

Host: gathers (indexing by matches/sel is part of input sharding), final
8-way sum of the 3 partials. The hardest-negative terms are exactly zero
whenever every row-min distance exceeds NEG_THRESH (true with huge margin for
this problem's data: min distance ~2.9 vs 1.4), in which case the pair-mask
cannot affect the result. If a nonzero negative sum is ever observed the
kernel falls back to an exact host recomputation (mask included).
"""

import numpy as np

import concourse.bacc as bacc
import concourse.mybir as mybir
import concourse.tile as tile
from concourse.bass_utils import run_bass_kernel_spmd

N_CORES = 8
N_PTS = 100000
D = 32
P = 16384
M = 8192
P_LOC = P // N_CORES            # 2048 rows per core
RT = P_LOC // 128               # 16 row tiles
CHUNK = 1024                    # q columns per PSUM tile (2 banks)
NCH = M // CHUNK                # 8 chunks
KA = D + 1                      # contraction dim with augmentation row
POS_THRESH = 0.1
NEG_THRESH = 1.4

F32 = mybir.dt.float32
AX = mybir.AxisListType
ALU = mybir.AluOpType
ACT = mybir.ActivationFunctionType

_CACHED_NC = None
LAST_RESULTS = None            # test.py reads .exec_time_ns after a traced run


def _register_const(nc, value):
    t = nc.alloc_sbuf_tensor(f"const-float32-{value}", [128, 1], F32)
    nc.gpsimd.memset(t.ap(), value)
    nc.const_aps.aps[(F32, value)] = t.ap()


def _build_nc():
    nc = bacc.Bacc("TRN2", debug=False, target_bir_lowering=False,
                   num_devices=N_CORES)
    for v in (-POS_THRESH, 1e-7, NEG_THRESH):
        _register_const(nc, v)
    nc.all_engine_barrier()
    lhsA = nc.dram_tensor("lhsA", [KA, P_LOC], F32, kind="ExternalInput").ap()
    lhsB = nc.dram_tensor("lhsB", [KA, P_LOC], F32, kind="ExternalInput").ap()
    rhsA = nc.dram_tensor("rhsA", [KA, M], F32, kind="ExternalInput").ap()
    rhsB = nc.dram_tensor("rhsB", [KA, M], F32, kind="ExternalInput").ap()
    pnA = nc.dram_tensor("pnA", [128, RT], F32, kind="ExternalInput").ap()
    pnB = nc.dram_tensor("pnB", [128, RT], F32, kind="ExternalInput").ap()
    ones = nc.dram_tensor("ones", [128, 1], F32, kind="ExternalInput").ap()
    outd = nc.dram_tensor("out", [1, 3], F32, kind="ExternalOutput").ap()

    with tile.TileContext(nc) as tc:
        with (
            tc.tile_pool(name="ops", bufs=1) as ops,
            tc.tile_pool(name="wk", bufs=2) as wk,
            tc.tile_pool(name="ps", bufs=3, space="PSUM") as ps,
            tc.tile_pool(name="psp", bufs=2, space="PSUM") as psp,
        ):
            t_lhsA = ops.tile([KA, P_LOC], F32, tag="lhsA")
            t_lhsB = ops.tile([KA, P_LOC], F32, tag="lhsB")
            t_rhsA = ops.tile([KA, M], F32, tag="rhsA")
            t_rhsB = ops.tile([KA, M], F32, tag="rhsB")
            t_pnA = ops.tile([128, RT], F32, tag="pnA")
            t_pnB = ops.tile([128, RT], F32, tag="pnB")
            t_ones = ops.tile([128, 1], F32, tag="ones")

            nc.sync.dma_start(t_lhsA[:], lhsA[:])
            nc.sync.dma_start(t_lhsB[:], lhsB[:])
            nc.sync.dma_start(t_pnA[:], pnA[:])
            nc.sync.dma_start(t_pnB[:], pnB[:])
            nc.sync.dma_start(t_ones[:], ones[:])
            # chunked loads so early matmuls can start before the full 1MB lands
            for k in range(NCH):
                sl = slice(k * CHUNK, (k + 1) * CHUNK)
                nc.sync.dma_start(t_rhsA[:, sl], rhsA[:, sl])
            for k in range(NCH):
                sl = slice(k * CHUNK, (k + 1) * CHUNK)
                nc.sync.dma_start(t_rhsB[:, sl], rhsB[:, sl])

            # ---- positive-pair loss: relu(sum_d (p0-p1)^2 - 0.1), summed ----
            t_dif = ops.tile([KA, P_LOC], F32, tag="dif")
            nc.vector.tensor_tensor(t_dif[:], t_lhsA[:], t_lhsB[:], ALU.subtract)
            t_difsq = ops.tile([KA, P_LOC], F32, tag="difsq")
            nc.scalar.activation(t_difsq[:], t_dif[:], ACT.Square)
            t_posr = ops.tile([1, P_LOC], F32, tag="posr")
            for j in range(P_LOC // 512):
                pp = psp.tile([1, 512], F32, tag="pp")
                nc.tensor.matmul(pp[:], t_ones[0:KA, 0:1],
                                 t_difsq[:, j * 512:(j + 1) * 512])
                nc.scalar.activation(t_posr[0:1, j * 512:(j + 1) * 512], pp[:],
                                     ACT.Relu, bias=-POS_THRESH)

            # ---- the two distance matrices: column-min per row ----
            t_cminA = ops.tile([128, RT * NCH], F32, tag="cminA")
            t_cminB = ops.tile([128, RT * NCH], F32, tag="cminB")
            for t_lhs, t_rhs, t_cmin in (
                (t_lhsA, t_rhsA, t_cminA),
                (t_lhsB, t_rhsB, t_cminB),
            ):
                for r in range(RT):
                    w = t_lhs[:, r * 128:(r + 1) * 128]
                    for k in range(NCH):
                        q = ps.tile([128, CHUNK], F32, tag="q")
                        for j in range(CHUNK // 512):
                            c0 = k * CHUNK + j * 512
                            nc.tensor.matmul(q[:, j * 512:(j + 1) * 512], w,
                                             t_rhs[:, c0:c0 + 512])
                        nc.vector.tensor_reduce(
                            out=t_cmin[:, r * NCH + k: r * NCH + k + 1],
                            in_=q[:], axis=AX.X, op=ALU.min)

            # ---- epilogue: dist -> relu(1.4 - dist)^2 -> row sums ----
            t_outsb = wk.tile([1, 3], F32, tag="outsb")
            nc.vector.tensor_reduce(out=t_outsb[0:1, 0:1], in_=t_posr[:],
                                    axis=AX.X, op=ALU.add)
            for idx, (t_cmin, t_pn) in enumerate(
                    ((t_cminA, t_pnA), (t_cminB, t_pnB))):
                minq = wk.tile([128, RT], F32, tag="minq")
                nc.vector.tensor_reduce(
                    out=minq[:],
                    in_=t_cmin.rearrange("p (r k) -> p r k", k=NCH),
                    axis=AX.X, op=ALU.min)
                d2 = wk.tile([128, RT], F32, tag="d2")
                nc.vector.tensor_tensor(d2[:], minq[:], t_pn[:], ALU.add)
                d2c = wk.tile([128, RT], F32, tag="d2c")
                nc.scalar.activation(d2c[:], d2[:], ACT.Relu)
                dist = wk.tile([128, RT], F32, tag="dist")
                nc.scalar.activation(dist[:], d2c[:], ACT.Sqrt, bias=1e-7)
                y = wk.tile([128, RT], F32, tag="y")
                nc.scalar.activation(y[:], dist[:], ACT.Relu,
                                     bias=NEG_THRESH, scale=-1.0)
                n2 = wk.tile([128, RT], F32, tag="n2")
                nc.scalar.activation(n2[:], y[:], ACT.Square)
                sm = wk.tile([128, 1], F32, tag="sm")
                nc.vector.tensor_reduce(out=sm[:], in_=n2[:], axis=AX.X,
                                        op=ALU.add)
                fp = psp.tile([1, 1], F32, tag="pp")
                nc.tensor.matmul(fp[:], sm[:], t_ones[:])
                nc.scalar.copy(t_outsb[0:1, idx + 1:idx + 2], fp[0:1, 0:1])

            nc.sync.dma_start(outd[:], t_outsb[:])

    nc.compile()
    return nc


def _prep_inputs(F0, F1, matches, sel0, sel1):
    posF0 = F0[matches[:, 0]]
    posF1 = F1[matches[:, 1]]
    subF0 = F0[sel0]
    subF1 = F1[sel1]
    ones_col = np.ones((1, P_LOC), np.float32)
    rhsA = np.ascontiguousarray(
        np.concatenate([-2.0 * subF1.T, (subF1 * subF1).sum(1)[None, :]], 0),
        dtype=np.float32)
    rhsB = np.ascontiguousarray(
        np.concatenate([-2.0 * subF0.T, (subF0 * subF0).sum(1)[None, :]], 0),
        dtype=np.float32)
    ones_in = np.ones((128, 1), np.float32)
    in_maps = []
    for c in range(N_CORES):
        sl = slice(c * P_LOC, (c + 1) * P_LOC)
        p0, p1 = posF0[sl], posF1[sl]
        in_maps.append({
            "lhsA": np.ascontiguousarray(
                np.concatenate([p0.T, ones_col], 0), dtype=np.float32),
            "lhsB": np.ascontiguousarray(
                np.concatenate([p1.T, ones_col], 0), dtype=np.float32),
            "rhsA": rhsA,
            "rhsB": rhsB,
            "pnA": np.ascontiguousarray(
                (p0 * p0).sum(1).reshape(RT, 128).T, dtype=np.float32),
            "pnB": np.ascontiguousarray(
                (p1 * p1).sum(1).reshape(RT, 128).T, dtype=np.float32),
            "ones": ones_in,
        })
    return in_maps


def _exact_host_reference(F0, F1, matches, sel0, sel1):
    """Bit-faithful numpy port of the oracle, used only as a fallback when a
    nonzero hardest-negative sum is observed (mask handling then matters)."""
    hash_seed = max(F0.shape[0], F1.shape[0])
    pos_ind0 = matches[:, 0].astype(np.int64)
    pos_ind1 = matches[:, 1].astype(np.int64)
    posF0, posF1 = F0[pos_ind0], F1[pos_ind1]
    subF0, subF1 = F0[sel0], F1[sel1]

    def pd(A, B):
        d2 = ((A * A).sum(1)[:, None] + (B * B).sum(1)[None, :]
              - 2.0 * (A @ B.T))
        return np.sqrt(np.maximum(d2, 0.0) + 1e-7)

    D01 = pd(posF0, subF1)
    D10 = pd(posF1, subF0)
    D01min, D10min = D01.min(1), D10.min(1)
    D01ind = np.asarray(sel1)[np.argmin(D01, 1)].astype(np.int64)
    D10ind = np.asarray(sel0)[np.argmin(D10, 1)].astype(np.int64)
    pos_keys = pos_ind0 + pos_ind1 * hash_seed
    mask0 = ~np.isin(pos_ind0 + D01ind * hash_seed, pos_keys)
    mask1 = ~np.isin(D10ind + pos_ind1 * hash_seed, pos_keys)
    pos_loss = np.mean(np.maximum(((posF0 - posF1) ** 2).sum(1) - POS_THRESH, 0))
    n0 = np.maximum(NEG_THRESH - D01min, 0) ** 2
    n1 = np.maximum(NEG_THRESH - D10min, 0) ** 2
    neg0 = (n0 * mask0).sum() / max(mask0.sum(), 1)
    neg1 = (n1 * mask1).sum() / max(mask1.sum(), 1)
    return np.float32(pos_loss + (neg0 + neg1) / 2.0)


def kernel(F0, F1, matches, sel0, sel1):
    global _CACHED_NC, LAST_RESULTS
    F0 = np.ascontiguousarray(np.asarray(F0), dtype=np.float32)
    F1 = np.ascontiguousarray(np.asarray(F1), dtype=np.float32)
    matches = np.asarray(matches)
    sel0 = np.asarray(sel0)
    sel1 = np.asarray(sel1)
    assert F0.shape == (N_PTS, D) and matches.shape == (P, 2)
    assert sel0.shape == (M,) and sel1.shape == (M,)

    in_maps = _prep_inputs(F0, F1, matches, sel0, sel1)
    if _CACHED_NC is None:
        _CACHED_NC = _build_nc()
    try:
        res = run_bass_kernel_spmd(_CACHED_NC, in_maps, list(range(N_CORES)))
    except Exception:
        # a wedged NeuronCore (e.g. NRT_EXEC_UNIT_UNRECOVERABLE from an
        # earlier crashed session) is recoverable via the axon reset call
        try:
            import ctypes

            lib = ctypes.CDLL("/opt/axon/libaxon_pjrt.so")
            lib.axon_reset.restype = ctypes.c_int64
            lib.axon_reset()
        except Exception:
            pass
        res = run_bass_kernel_spmd(_CACHED_NC, in_maps, list(range(N_CORES)))
    LAST_RESULTS = res
    outs = np.stack([r["out"] for r in res.results])   # (8, 1, 3)
    pos_sum = float(outs[:, 0, 0].sum())
    sA = float(outs[:, 0, 1].sum())
    sB = float(outs[:, 0, 2].sum())
    if sA != 0.0 or sB != 0.0:
        # hardest negatives crossed NEG_THRESH: the pair-mask now matters.
        return _exact_host_reference(F0, F1, matches, sel0, sel1)
    return np.float32(pos_sum / P)


# revision 8
# speedup vs baseline: 2.7767x; 2.7767x over previous
"""ContrastiveHardestNegativeLoss on 8 Trainium2 NeuronCores (Bass/Tile).

Strategy (per sharding hint): shard the positive-pair (row) dimension of the
P x M distance matrices across the 8 cores. Each core receives:
  - its slice of the gathered pos features, transposed + augmented:
      lhs[d, i] = posF[i, d] for d < 32,  lhs[32, i] = 1.0
  - the full gathered sub features, transposed, scaled and augmented:
      rhs[d, c] = -2 * subF[c, d] for d < 32,  rhs[32, c] = |subF[c]|^2
  so a single PE matmul produces q[i, c] = |subF[c]|^2 - 2 <posF[i], subF[c]>,
  i.e. the squared distance minus the (row-constant) |posF[i]|^2 term, which is
  added back after the column-min reduction (min is invariant to it).

Device per core: 2 matrices x 16 row-tiles x 8 col-chunks of matmul (K=33,
N=512) into PSUM, VectorE min-reduce per chunk, then a tiny epilogue:
dist = sqrt(max(pn + min_q, 0) + 1e-7), n = relu(1.4 - dist)^2, summed across
rows on the PE (ones-matmul). The positive loss relu(|p0-p1|^2 - 0.1) is
computed from the same augmented operands. Output: 3 partial sums per core.

Host: gathers (indexing by matches/sel is part of input sharding), final
8-way sum of the 3 partials. The hardest-negative terms are exactly zero
whenever every row-min distance exceeds NEG_THRESH (true with huge margin for
this problem's data: min distance ~2.9 vs 1.4), in which case the pair-mask
cannot affect the result. If a nonzero negative sum is ever observed the
kernel falls back to an exact host recomputation (mask included).
"""

import numpy as np

import concourse.bacc as bacc
import concourse.mybir as mybir
import concourse.tile as tile
from concourse.bass_utils import run_bass_kernel_spmd

N_CORES = 8
N_PTS = 100000
D = 32
P = 16384
M = 8192
P_LOC = P // N_CORES            # 2048 rows per core
RT = P_LOC // 128               # 16 row tiles
CHUNK = 1024                    # q columns per PSUM tile (2 banks)
NCH = M // CHUNK                # 8 chunks
KA = D + 1                      # contraction dim with augmentation row
POS_THRESH = 0.1
NEG_THRESH = 1.4

F32 = mybir.dt.float32
BF16 = mybir.dt.bfloat16
AX = mybir.AxisListType
ALU = mybir.AluOpType
ACT = mybir.ActivationFunctionType

_CACHED_NC = None
LAST_RESULTS = None            # test.py reads .exec_time_ns after a traced run


def _register_const(nc, value):
    t = nc.alloc_sbuf_tensor(f"const-float32-{value}", [128, 1], F32)
    nc.gpsimd.memset(t.ap(), value)
    nc.const_aps.aps[(F32, value)] = t.ap()


def _build_nc():
    nc = bacc.Bacc("TRN2", debug=False, target_bir_lowering=False,
                   num_devices=N_CORES)
    for v in (-POS_THRESH, 1e-7, NEG_THRESH):
        _register_const(nc, v)
    nc.all_engine_barrier()
    # fp32 pos-pair operands (positive loss needs full precision);
    # bf16 copies feed the distance-matrix matmuls (fp32 PE matmul streams at
    # 1/4 rate; bf16 error on a distance is ~0.05 vs a 1.5 threshold margin).
    lhsA = nc.dram_tensor("lhsA", [KA, P_LOC], F32, kind="ExternalInput").ap()
    lhsB = nc.dram_tensor("lhsB", [KA, P_LOC], F32, kind="ExternalInput").ap()
    lhsAh = nc.dram_tensor("lhsAh", [KA, P_LOC], BF16, kind="ExternalInput").ap()
    lhsBh = nc.dram_tensor("lhsBh", [KA, P_LOC], BF16, kind="ExternalInput").ap()
    rhsAh = nc.dram_tensor("rhsAh", [KA, M], BF16, kind="ExternalInput").ap()
    rhsBh = nc.dram_tensor("rhsBh", [KA, M], BF16, kind="ExternalInput").ap()
    pnA = nc.dram_tensor("pnA", [128, RT], F32, kind="ExternalInput").ap()
    pnB = nc.dram_tensor("pnB", [128, RT], F32, kind="ExternalInput").ap()
    ones = nc.dram_tensor("ones", [128, 1], F32, kind="ExternalInput").ap()
    outd = nc.dram_tensor("out", [1, 3], F32, kind="ExternalOutput").ap()

    with tile.TileContext(nc) as tc:
        with (
            tc.tile_pool(name="ops", bufs=1) as ops,
            tc.tile_pool(name="wk", bufs=2) as wk,
            tc.tile_pool(name="ps", bufs=3, space="PSUM") as ps,
            tc.tile_pool(name="psp", bufs=2, space="PSUM") as psp,
        ):
            t_lhsA = ops.tile([KA, P_LOC], F32, tag="lhsA")
            t_lhsB = ops.tile([KA, P_LOC], F32, tag="lhsB")
            t_lhsAh = ops.tile([KA, P_LOC], BF16, tag="lhsAh")
            t_lhsBh = ops.tile([KA, P_LOC], BF16, tag="lhsBh")
            t_rhsAh = ops.tile([KA, M], BF16, tag="rhsAh")
            t_rhsBh = ops.tile([KA, M], BF16, tag="rhsBh")
            t_pnA = ops.tile([128, RT], F32, tag="pnA")
            t_pnB = ops.tile([128, RT], F32, tag="pnB")
            t_ones = ops.tile([128, 1], F32, tag="ones")

            nc.sync.dma_start(t_lhsA[:], lhsA[:])
            nc.sync.dma_start(t_lhsB[:], lhsB[:])
            nc.sync.dma_start(t_lhsAh[:], lhsAh[:])
            nc.sync.dma_start(t_lhsBh[:], lhsBh[:])
            nc.sync.dma_start(t_pnA[:], pnA[:])
            nc.sync.dma_start(t_pnB[:], pnB[:])
            nc.sync.dma_start(t_ones[:], ones[:])
            # chunked loads so early matmuls can start before the full load lands
            for k in range(NCH):
                sl = slice(k * CHUNK, (k + 1) * CHUNK)
                nc.sync.dma_start(t_rhsAh[:, sl], rhsAh[:, sl])
            for k in range(NCH):
                sl = slice(k * CHUNK, (k + 1) * CHUNK)
                nc.sync.dma_start(t_rhsBh[:, sl], rhsBh[:, sl])

            # ---- positive-pair loss: relu(sum_d (p0-p1)^2 - 0.1), summed ----
            t_dif = ops.tile([KA, P_LOC], F32, tag="dif")
            nc.vector.tensor_tensor(t_dif[:], t_lhsA[:], t_lhsB[:], ALU.subtract)
            t_difsq = ops.tile([KA, P_LOC], F32, tag="difsq")
            nc.scalar.activation(t_difsq[:], t_dif[:], ACT.Square)
            t_posr = ops.tile([1, P_LOC], F32, tag="posr")
            for j in range(P_LOC // 512):
                pp = psp.tile([1, 512], F32, tag="pp")
                nc.tensor.matmul(pp[:], t_ones[0:KA, 0:1],
                                 t_difsq[:, j * 512:(j + 1) * 512])
                nc.scalar.activation(t_posr[0:1, j * 512:(j + 1) * 512], pp[:],
                                     ACT.Relu, bias=-POS_THRESH)

            # ---- the two distance matrices: column-min per row ----
            t_cminA = ops.tile([128, RT * NCH], F32, tag="cminA")
            t_cminB = ops.tile([128, RT * NCH], F32, tag="cminB")
            for t_lhs, t_rhs, t_cmin in (
                (t_lhsAh, t_rhsAh, t_cminA),
                (t_lhsBh, t_rhsBh, t_cminB),
            ):
                for r in range(RT):
                    w = t_lhs[:, r * 128:(r + 1) * 128]
                    for k in range(NCH):
                        q = ps.tile([128, CHUNK], F32, tag="q")
                        for j in range(CHUNK // 512):
                            c0 = k * CHUNK + j * 512
                            nc.tensor.matmul(q[:, j * 512:(j + 1) * 512], w,
                                             t_rhs[:, c0:c0 + 512])
                        nc.vector.tensor_reduce(
                            out=t_cmin[:, r * NCH + k: r * NCH + k + 1],
                            in_=q[:], axis=AX.X, op=ALU.min)

            # ---- epilogue: dist -> relu(1.4 - dist)^2 -> row sums ----
            t_outsb = wk.tile([1, 3], F32, tag="outsb")
            nc.vector.tensor_reduce(out=t_outsb[0:1, 0:1], in_=t_posr[:],
                                    axis=AX.X, op=ALU.add)
            for idx, (t_cmin, t_pn) in enumerate(
                    ((t_cminA, t_pnA), (t_cminB, t_pnB))):
                minq = wk.tile([128, RT], F32, tag="minq")
                nc.vector.tensor_reduce(
                    out=minq[:],
                    in_=t_cmin.rearrange("p (r k) -> p r k", k=NCH),
                    axis=AX.X, op=ALU.min)
                d2 = wk.tile([128, RT], F32, tag="d2")
                nc.vector.tensor_tensor(d2[:], minq[:], t_pn[:], ALU.add)
                d2c = wk.tile([128, RT], F32, tag="d2c")
                nc.scalar.activation(d2c[:], d2[:], ACT.Relu)
                dist = wk.tile([128, RT], F32, tag="dist")
                nc.scalar.activation(dist[:], d2c[:], ACT.Sqrt, bias=1e-7)
                y = wk.tile([128, RT], F32, tag="y")
                nc.scalar.activation(y[:], dist[:], ACT.Relu,
                                     bias=NEG_THRESH, scale=-1.0)
                n2 = wk.tile([128, RT], F32, tag="n2")
                nc.scalar.activation(n2[:], y[:], ACT.Square)
                sm = wk.tile([128, 1], F32, tag="sm")
                nc.vector.tensor_reduce(out=sm[:], in_=n2[:], axis=AX.X,
                                        op=ALU.add)
                fp = psp.tile([1, 1], F32, tag="pp")
                nc.tensor.matmul(fp[:], sm[:], t_ones[:])
                nc.scalar.copy(t_outsb[0:1, idx + 1:idx + 2], fp[0:1, 0:1])

            nc.sync.dma_start(outd[:], t_outsb[:])

    nc.compile()
    return nc


def _prep_inputs(F0, F1, matches, sel0, sel1):
    posF0 = F0[matches[:, 0]]
    posF1 = F1[matches[:, 1]]
    subF0 = F0[sel0]
    subF1 = F1[sel1]
    import ml_dtypes

    bf16 = ml_dtypes.bfloat16
    ones_col = np.ones((1, P_LOC), np.float32)
    rhsA = np.ascontiguousarray(
        np.concatenate([-2.0 * subF1.T, (subF1 * subF1).sum(1)[None, :]], 0),
        dtype=np.float32)
    rhsB = np.ascontiguousarray(
        np.concatenate([-2.0 * subF0.T, (subF0 * subF0).sum(1)[None, :]], 0),
        dtype=np.float32)
    rhsAh = np.ascontiguousarray(rhsA, dtype=bf16)
    rhsBh = np.ascontiguousarray(rhsB, dtype=bf16)
    ones_in = np.ones((128, 1), np.float32)
    in_maps = []
    for c in range(N_CORES):
        sl = slice(c * P_LOC, (c + 1) * P_LOC)
        p0, p1 = posF0[sl], posF1[sl]
        lhsA = np.ascontiguousarray(
            np.concatenate([p0.T, ones_col], 0), dtype=np.float32)
        lhsB = np.ascontiguousarray(
            np.concatenate([p1.T, ones_col], 0), dtype=np.float32)
        in_maps.append({
            "lhsA": lhsA,
            "lhsB": lhsB,
            "lhsAh": np.ascontiguousarray(lhsA, dtype=bf16),
            "lhsBh": np.ascontiguousarray(lhsB, dtype=bf16),
            "rhsAh": rhsAh,
            "rhsBh": rhsBh,
            "pnA": np.ascontiguousarray(
                (p0 * p0).sum(1).reshape(RT, 128).T, dtype=np.float32),
            "pnB": np.ascontiguousarray(
                (p1 * p1).sum(1).reshape(RT, 128).T, dtype=np.float32),
            "ones": ones_in,
        })
    return in_maps


def _exact_host_reference(F0, F1, matches, sel0, sel1):
    """Bit-faithful numpy port of the oracle, used only as a fallback when a
    nonzero hardest-negative sum is observed (mask handling then matters)."""
    hash_seed = max(F0.shape[0], F1.shape[0])
    pos_ind0 = matches[:, 0].astype(np.int64)
    pos_ind1 = matches[:, 1].astype(np.int64)
    posF0, posF1 = F0[pos_ind0], F1[pos_ind1]
    subF0, subF1 = F0[sel0], F1[sel1]

    def pd(A, B):
        d2 = ((A * A).sum(1)[:, None] + (B * B).sum(1)[None, :]
              - 2.0 * (A @ B.T))
        return np.sqrt(np.maximum(d2, 0.0) + 1e-7)

    D01 = pd(posF0, subF1)
    D10 = pd(posF1, subF0)
    D01min, D10min = D01.min(1), D10.min(1)
    D01ind = np.asarray(sel1)[np.argmin(D01, 1)].astype(np.int64)
    D10ind = np.asarray(sel0)[np.argmin(D10, 1)].astype(np.int64)
    pos_keys = pos_ind0 + pos_ind1 * hash_seed
    mask0 = ~np.isin(pos_ind0 + D01ind * hash_seed, pos_keys)
    mask1 = ~np.isin(D10ind + pos_ind1 * hash_seed, pos_keys)
    pos_loss = np.mean(np.maximum(((posF0 - posF1) ** 2).sum(1) - POS_THRESH, 0))
    n0 = np.maximum(NEG_THRESH - D01min, 0) ** 2
    n1 = np.maximum(NEG_THRESH - D10min, 0) ** 2
    neg0 = (n0 * mask0).sum() / max(mask0.sum(), 1)
    neg1 = (n1 * mask1).sum() / max(mask1.sum(), 1)
    return np.float32(pos_loss + (neg0 + neg1) / 2.0)


def kernel(F0, F1, matches, sel0, sel1):
    global _CACHED_NC, LAST_RESULTS
    F0 = np.ascontiguousarray(np.asarray(F0), dtype=np.float32)
    F1 = np.ascontiguousarray(np.asarray(F1), dtype=np.float32)
    matches = np.asarray(matches)
    sel0 = np.asarray(sel0)
    sel1 = np.asarray(sel1)
    assert F0.shape == (N_PTS, D) and matches.shape == (P, 2)
    assert sel0.shape == (M,) and sel1.shape == (M,)

    in_maps = _prep_inputs(F0, F1, matches, sel0, sel1)
    if _CACHED_NC is None:
        _CACHED_NC = _build_nc()
    try:
        res = run_bass_kernel_spmd(_CACHED_NC, in_maps, list(range(N_CORES)))
    except Exception:
        # a wedged NeuronCore (e.g. NRT_EXEC_UNIT_UNRECOVERABLE from an
        # earlier crashed session) is recoverable via the axon reset call
        try:
            import ctypes

            lib = ctypes.CDLL("/opt/axon/libaxon_pjrt.so")
            lib.axon_reset.restype = ctypes.c_int64
            lib.axon_reset()
        except Exception:
            pass
        res = run_bass_kernel_spmd(_CACHED_NC, in_maps, list(range(N_CORES)))
    LAST_RESULTS = res
    outs = np.stack([r["out"] for r in res.results])   # (8, 1, 3)
    pos_sum = float(outs[:, 0, 0].sum())
    sA = float(outs[:, 0, 1].sum())
    sB = float(outs[:, 0, 2].sum())
    if sA != 0.0 or sB != 0.0:
        # hardest negatives crossed NEG_THRESH: the pair-mask now matters.
        return _exact_host_reference(F0, F1, matches, sel0, sel1)
    return np.float32(pos_sum / P)


# revision 15
# speedup vs baseline: 3.5024x; 1.2614x over previous
"""ContrastiveHardestNegativeLoss on 8 Trainium2 NeuronCores (Bass/Tile).

Strategy (per sharding hint): shard the positive-pair (row) dimension of the
P x M distance matrices across the 8 cores. Each core receives:
  - its slice of the gathered pos features, transposed + augmented:
      lhs[d, i] = posF[i, d] for d < 32,  lhs[32, i] = 1.0
  - the full gathered sub features, transposed, scaled and augmented:
      rhs[d, c] = -2 * subF[c, d] for d < 32,  rhs[32, c] = |subF[c]|^2
  so a single PE matmul produces q[i, c] = |subF[c]|^2 - 2 <posF[i], subF[c]>,
  i.e. the squared distance minus the (row-constant) |posF[i]|^2 term, which is
  added back after the column-min reduction (min is invariant to it).

Device per core: 2 matrices x 16 row-tiles x 8 col-chunks of matmul (K=33,
N=512) into PSUM, VectorE min-reduce per chunk, then a tiny epilogue:
dist = sqrt(max(pn + min_q, 0) + 1e-7), n = relu(1.4 - dist)^2, summed across
rows on the PE (ones-matmul). The positive loss relu(|p0-p1|^2 - 0.1) is
computed from the same augmented operands. Output: 3 partial sums per core.

Host: gathers (indexing by matches/sel is part of input sharding), final
8-way sum of the 3 partials. The hardest-negative terms are exactly zero
whenever every row-min distance exceeds NEG_THRESH (true with huge margin for
this problem's data: min distance ~2.9 vs 1.4), in which case the pair-mask
cannot affect the result. If a nonzero negative sum is ever observed the
kernel falls back to an exact host recomputation (mask included).
"""

import numpy as np

import concourse.bacc as bacc
import concourse.mybir as mybir
import concourse.tile as tile
from concourse.bass_utils import run_bass_kernel_spmd

N_CORES = 8
N_PTS = 100000
D = 32
P = 16384
M = 8192
P_LOC = P // N_CORES            # 2048 rows per core
RT = P_LOC // 128               # 16 row tiles
CHUNK = 1024                    # q columns per PSUM tile (2 banks)
NCH = M // CHUNK                # 8 chunks
KA = D + 1                      # contraction dim with augmentation row
POS_THRESH = 0.1
NEG_THRESH = 1.4

F32 = mybir.dt.float32
BF16 = mybir.dt.bfloat16
AX = mybir.AxisListType
ALU = mybir.AluOpType
ACT = mybir.ActivationFunctionType

_CACHED_NC = None
LAST_RESULTS = None            # test.py reads .exec_time_ns after a traced run


def _register_const(nc, value):
    t = nc.alloc_sbuf_tensor(f"const-float32-{value}", [128, 1], F32)
    nc.gpsimd.memset(t.ap(), value)
    nc.const_aps.aps[(F32, value)] = t.ap()


def _register_min2():
    """Custom DVE op: out = min(in0, in1) elementwise, accum_out[p] =
    min(s0, min_k out[p, k]). Consumes TWO streams per cycle (rd0 + rd1),
    doubling reduction throughput vs stock tensor_reduce (which is capped at
    one element/lane/cycle). Registered at runtime into dve_ops.OPS so the
    per-NEFF DVE table generator can resolve it by name."""
    import concourse.dve_ops as dops
    from concourse.dve_spec import C0, Spec, Src0, Src1, _has_src1, lower, minn
    from concourse.dve_uop import DveOpSpec

    name = "MIN2_STREAMS_ANT"
    for op in dops.OPS:
        if op.name == name:
            return op

    def ref(in0, in1, s0, s1, imm2):
        b = np.minimum(in0, in1).astype(np.float32)
        acc = np.minimum(b.reshape(b.shape[0], -1).min(-1, keepdims=True),
                         np.asarray(s0, np.float32).reshape(-1, 1))
        return b, acc

    spec = Spec(body=minn(Src0, Src1), accum=minn, accum_init=C0, reference=ref)
    row = dops._CUSTOM_DVE_ROW_BASE + len(dops.OPS)
    shas = {}
    for ver in ("v3", "v4"):
        uops = lower(spec, ver=ver)
        shas[ver] = DveOpSpec(name=name, opcode=row, uops=uops,
                              rd1_en=_has_src1(spec)).sha(ver)
    op = dops.DveOp(name, spec, subdim=False, uops_sha=shas)
    dops.OPS.append(op)
    dops.CUSTOM_DVE_SPECS[name] = spec
    dops._SUB_OPCODE_FOR_NAME[name] = row
    return op


def _build_nc():
    min2 = _register_min2()
    nc = bacc.Bacc("TRN2", debug=False, target_bir_lowering=False,
                   num_devices=N_CORES)
    for v in (-POS_THRESH, 1e-7, NEG_THRESH):
        _register_const(nc, v)
    nc.all_engine_barrier()
    # fp32 pos-pair operands (positive loss needs full precision);
    # bf16 copies feed the distance-matrix matmuls (fp32 PE matmul streams at
    # 1/4 rate; bf16 error on a distance is ~0.05 vs a 1.5 threshold margin).
    lhsA = nc.dram_tensor("lhsA", [KA, P_LOC], F32, kind="ExternalInput").ap()
    lhsB = nc.dram_tensor("lhsB", [KA, P_LOC], F32, kind="ExternalInput").ap()
    lhsAh = nc.dram_tensor("lhsAh", [KA, P_LOC], BF16, kind="ExternalInput").ap()
    lhsBh = nc.dram_tensor("lhsBh", [KA, P_LOC], BF16, kind="ExternalInput").ap()
    rhsAh = nc.dram_tensor("rhsAh", [KA, M], BF16, kind="ExternalInput").ap()
    rhsBh = nc.dram_tensor("rhsBh", [KA, M], BF16, kind="ExternalInput").ap()
    pnA = nc.dram_tensor("pnA", [128, RT], F32, kind="ExternalInput").ap()
    pnB = nc.dram_tensor("pnB", [128, RT], F32, kind="ExternalInput").ap()
    ones = nc.dram_tensor("ones", [128, 1], F32, kind="ExternalInput").ap()
    outd = nc.dram_tensor("out", [1, 3], F32, kind="ExternalOutput").ap()

    with tile.TileContext(nc) as tc:
        with (
            tc.tile_pool(name="ops", bufs=1) as ops,
            tc.tile_pool(name="wk", bufs=2) as wk,
            tc.tile_pool(name="ps", bufs=4, space="PSUM") as ps,
        ):
            t_lhsA = ops.tile([KA, P_LOC], F32, tag="lhsA")
            t_lhsB = ops.tile([KA, P_LOC], F32, tag="lhsB")
            # bf16 operands are loaded TWICE: rows 0..32 and rows 64..96, so
            # two row-tiles' matmuls can run concurrently on the two 64-row
            # groups of the PE array (K=33 rounds up to a 64-row group).
            t_lhsAh = ops.tile([128, P_LOC], BF16, tag="lhsAh")
            t_lhsBh = ops.tile([128, P_LOC], BF16, tag="lhsBh")
            t_rhsAh = ops.tile([128, M], BF16, tag="rhsAh")
            t_rhsBh = ops.tile([128, M], BF16, tag="rhsBh")
            t_pnA = ops.tile([128, RT], F32, tag="pnA")
            t_pnB = ops.tile([128, RT], F32, tag="pnB")
            t_ones = ops.tile([128, 1], F32, tag="ones")

            nc.sync.dma_start(t_lhsA[:], lhsA[:])
            nc.sync.dma_start(t_lhsB[:], lhsB[:])
            for base in (0, 64):
                nc.sync.dma_start(t_lhsAh[base:base + KA, :], lhsAh[:])
                nc.sync.dma_start(t_lhsBh[base:base + KA, :], lhsBh[:])
            nc.sync.dma_start(t_pnA[:], pnA[:])
            nc.sync.dma_start(t_pnB[:], pnB[:])
            nc.sync.dma_start(t_ones[:], ones[:])
            # chunked loads so early matmuls can start before the full load lands
            for t_rhs, rhs_d in ((t_rhsAh, rhsAh), (t_rhsBh, rhsBh)):
                for k in range(NCH):
                    sl = slice(k * CHUNK, (k + 1) * CHUNK)
                    for base in (0, 64):
                        nc.sync.dma_start(t_rhs[base:base + KA, sl], rhs_d[:, sl])

            # ---- positive-pair loss: relu(sum_d (p0-p1)^2 - 0.1), summed ----
            t_dif = ops.tile([KA, P_LOC], F32, tag="dif")
            nc.vector.tensor_tensor(t_dif[:], t_lhsA[:], t_lhsB[:], ALU.subtract)
            t_difsq = ops.tile([KA, P_LOC], F32, tag="difsq")
            nc.scalar.activation(t_difsq[:], t_dif[:], ACT.Square)
            t_posr = ops.tile([1, P_LOC], F32, tag="posr")
            for j in range(P_LOC // 512):
                pp = ps.tile([1, 512], F32, tag="q")
                nc.tensor.matmul(pp[:], t_ones[0:KA, 0:1],
                                 t_difsq[:, j * 512:(j + 1) * 512])
                nc.scalar.activation(t_posr[0:1, j * 512:(j + 1) * 512], pp[:],
                                     ACT.Relu, bias=-POS_THRESH)

            # ---- the two distance matrices: column-min per row ----
            # Row-tiles are processed in pairs (PE row-groups 0 and 64). Per
            # row-tile, chunk pairs (even PSUM, odd copied to SBUF by ScalarE)
            # feed the 2-stream custom DVE min, halving VectorE time.
            NPAIR = NCH // 2          # 4 min results per row tile
            t_cminA = ops.tile([128, RT * NPAIR], F32, tag="cminA")
            t_cminB = ops.tile([128, RT * NPAIR], F32, tag="cminB")
            for t_lhs, t_rhs, t_cmin in (
                (t_lhsAh, t_rhsAh, t_cminA),
                (t_lhsBh, t_rhsBh, t_cminB),
            ):
                for pr in range(RT // 2):
                    held = {}
                    for k in range(NCH):
                        for half in (0, 1):
                            r = 2 * pr + half
                            base = 64 * half
                            w = t_lhs[base:base + KA, r * 128:(r + 1) * 128]
                            q = ps.tile([128, CHUNK], F32, tag="q")
                            for j in range(CHUNK // 512):
                                c0 = k * CHUNK + j * 512
                                nc.tensor.matmul(
                                    q[:, j * 512:(j + 1) * 512], w,
                                    t_rhs[base:base + KA, c0:c0 + 512])
                            if k % 2 == 0:
                                held[half] = q
                            else:
                                qc = wk.tile([128, CHUNK], F32, tag="qc",
                                             bufs=3)
                                nc.scalar.copy(qc[:], q[:])
                                junk = wk.tile([128, CHUNK], F32, tag="junk",
                                               bufs=2)
                                col = r * NPAIR + k // 2
                                nc.vector._custom_dve(
                                    min2, out=junk[:], in0=held[half][:],
                                    in1=qc[:], s0=3.0e38,
                                    accum_out=t_cmin[:, col:col + 1])

            # ---- epilogue: dist -> relu(1.4 - dist)^2 -> row sums ----
            t_outsb = wk.tile([1, 3], F32, tag="outsb")
            nc.vector.tensor_reduce(out=t_outsb[0:1, 0:1], in_=t_posr[:],
                                    axis=AX.X, op=ALU.add)
            for idx, (t_cmin, t_pn) in enumerate(
                    ((t_cminA, t_pnA), (t_cminB, t_pnB))):
                minq = wk.tile([128, RT], F32, tag="minq")
                nc.vector.tensor_reduce(
                    out=minq[:],
                    in_=t_cmin.rearrange("p (r k) -> p r k", k=NPAIR),
                    axis=AX.X, op=ALU.min)
                d2 = wk.tile([128, RT], F32, tag="d2")
                nc.vector.tensor_tensor(d2[:], minq[:], t_pn[:], ALU.add)
                d2c = wk.tile([128, RT], F32, tag="d2c")
                nc.scalar.activation(d2c[:], d2[:], ACT.Relu)
                dist = wk.tile([128, RT], F32, tag="dist")
                nc.scalar.activation(dist[:], d2c[:], ACT.Sqrt, bias=1e-7)
                y = wk.tile([128, RT], F32, tag="y")
                nc.scalar.activation(y[:], dist[:], ACT.Relu,
                                     bias=NEG_THRESH, scale=-1.0)
                n2 = wk.tile([128, RT], F32, tag="n2")
                nc.scalar.activation(n2[:], y[:], ACT.Square)
                sm = wk.tile([128, 1], F32, tag="sm")
                nc.vector.tensor_reduce(out=sm[:], in_=n2[:], axis=AX.X,
                                        op=ALU.add)
                fp = ps.tile([1, 1], F32, tag="q")
                nc.tensor.matmul(fp[:], sm[:], t_ones[:])
                nc.scalar.copy(t_outsb[0:1, idx + 1:idx + 2], fp[0:1, 0:1])

            nc.sync.dma_start(outd[:], t_outsb[:])

    nc.compile()
    return nc


def _prep_inputs(F0, F1, matches, sel0, sel1):
    posF0 = F0[matches[:, 0]]
    posF1 = F1[matches[:, 1]]
    subF0 = F0[sel0]
    subF1 = F1[sel1]
    import ml_dtypes

    bf16 = ml_dtypes.bfloat16
    ones_col = np.ones((1, P_LOC), np.float32)
    rhsA = np.ascontiguousarray(
        np.concatenate([-2.0 * subF1.T, (subF1 * subF1).sum(1)[None, :]], 0),
        dtype=np.float32)
    rhsB = np.ascontiguousarray(
        np.concatenate([-2.0 * subF0.T, (subF0 * subF0).sum(1)[None, :]], 0),
        dtype=np.float32)
    rhsAh = np.ascontiguousarray(rhsA, dtype=bf16)
    rhsBh = np.ascontiguousarray(rhsB, dtype=bf16)
    ones_in = np.ones((128, 1), np.float32)
    in_maps = []
    for c in range(N_CORES):
        sl = slice(c * P_LOC, (c + 1) * P_LOC)
        p0, p1 = posF0[sl], posF1[sl]
        lhsA = np.ascontiguousarray(
            np.concatenate([p0.T, ones_col], 0), dtype=np.float32)
        lhsB = np.ascontiguousarray(
            np.concatenate([p1.T, ones_col], 0), dtype=np.float32)
        in_maps.append({
            "lhsA": lhsA,
            "lhsB": lhsB,
            "lhsAh": np.ascontiguousarray(lhsA, dtype=bf16),
            "lhsBh": np.ascontiguousarray(lhsB, dtype=bf16),
            "rhsAh": rhsAh,
            "rhsBh": rhsBh,
            "pnA": np.ascontiguousarray(
                (p0 * p0).sum(1).reshape(RT, 128).T, dtype=np.float32),
            "pnB": np.ascontiguousarray(
                (p1 * p1).sum(1).reshape(RT, 128).T, dtype=np.float32),
            "ones": ones_in,
        })
    return in_maps


def _exact_host_reference(F0, F1, matches, sel0, sel1):
    """Bit-faithful numpy port of the oracle, used only as a fallback when a
    nonzero hardest-negative sum is observed (mask handling then matters)."""
    hash_seed = max(F0.shape[0], F1.shape[0])
    pos_ind0 = matches[:, 0].astype(np.int64)
    pos_ind1 = matches[:, 1].astype(np.int64)
    posF0, posF1 = F0[pos_ind0], F1[pos_ind1]
    subF0, subF1 = F0[sel0], F1[sel1]

    def pd(A, B):
        d2 = ((A * A).sum(1)[:, None] + (B * B).sum(1)[None, :]
              - 2.0 * (A @ B.T))
        return np.sqrt(np.maximum(d2, 0.0) + 1e-7)

    D01 = pd(posF0, subF1)
    D10 = pd(posF1, subF0)
    D01min, D10min = D01.min(1), D10.min(1)
    D01ind = np.asarray(sel1)[np.argmin(D01, 1)].astype(np.int64)
    D10ind = np.asarray(sel0)[np.argmin(D10, 1)].astype(np.int64)
    pos_keys = pos_ind0 + pos_ind1 * hash_seed
    mask0 = ~np.isin(pos_ind0 + D01ind * hash_seed, pos_keys)
    mask1 = ~np.isin(D10ind + pos_ind1 * hash_seed, pos_keys)
    pos_loss = np.mean(np.maximum(((posF0 - posF1) ** 2).sum(1) - POS_THRESH, 0))
    n0 = np.maximum(NEG_THRESH - D01min, 0) ** 2
    n1 = np.maximum(NEG_THRESH - D10min, 0) ** 2
    neg0 = (n0 * mask0).sum() / max(mask0.sum(), 1)
    neg1 = (n1 * mask1).sum() / max(mask1.sum(), 1)
    return np.float32(pos_loss + (neg0 + neg1) / 2.0)


def kernel(F0, F1, matches, sel0, sel1):
    global _CACHED_NC, LAST_RESULTS
    F0 = np.ascontiguousarray(np.asarray(F0), dtype=np.float32)
    F1 = np.ascontiguousarray(np.asarray(F1), dtype=np.float32)
    matches = np.asarray(matches)
    sel0 = np.asarray(sel0)
    sel1 = np.asarray(sel1)
    assert F0.shape == (N_PTS, D) and matches.shape == (P, 2)
    assert sel0.shape == (M,) and sel1.shape == (M,)

    in_maps = _prep_inputs(F0, F1, matches, sel0, sel1)
    if _CACHED_NC is None:
        _CACHED_NC = _build_nc()
    try:
        res = run_bass_kernel_spmd(_CACHED_NC, in_maps, list(range(N_CORES)))
    except Exception:
        # a wedged NeuronCore (e.g. NRT_EXEC_UNIT_UNRECOVERABLE from an
        # earlier crashed session) is recoverable via the axon reset call
        try:
            import ctypes

            lib = ctypes.CDLL("/opt/axon/libaxon_pjrt.so")
            lib.axon_reset.restype = ctypes.c_int64
            lib.axon_reset()
        except Exception:
            pass
        res = run_bass_kernel_spmd(_CACHED_NC, in_maps, list(range(N_CORES)))
    LAST_RESULTS = res
    outs = np.stack([r["out"] for r in res.results])   # (8, 1, 3)
    pos_sum = float(outs[:, 0, 0].sum())
    sA = float(outs[:, 0, 1].sum())
    sB = float(outs[:, 0, 2].sum())
    if sA != 0.0 or sB != 0.0:
        # hardest negatives crossed NEG_THRESH: the pair-mask now matters.
        return _exact_host_reference(F0, F1, matches, sel0, sel1)
    return np.float32(pos_sum / P)


# revision 17
# speedup vs baseline: 3.5769x; 1.0213x over previous
"""ContrastiveHardestNegativeLoss on 8 Trainium2 NeuronCores (Bass/Tile).

Strategy (per sharding hint): shard the positive-pair (row) dimension of the
P x M distance matrices across the 8 cores. Each core receives:
  - its slice of the gathered pos features, transposed + augmented:
      lhs[d, i] = posF[i, d] for d < 32,  lhs[32, i] = 1.0
  - the full gathered sub features, transposed, scaled and augmented:
      rhs[d, c] = -2 * subF[c, d] for d < 32,  rhs[32, c] = |subF[c]|^2
  so a single PE matmul produces q[i, c] = |subF[c]|^2 - 2 <posF[i], subF[c]>,
  i.e. the squared distance minus the (row-constant) |posF[i]|^2 term, which is
  added back after the column-min reduction (min is invariant to it).

Device per core: 2 matrices x 16 row-tiles x 8 col-chunks of matmul (K=33,
N=512) into PSUM, VectorE min-reduce per chunk, then a tiny epilogue:
dist = sqrt(max(pn + min_q, 0) + 1e-7), n = relu(1.4 - dist)^2, summed across
rows on the PE (ones-matmul). The positive loss relu(|p0-p1|^2 - 0.1) is
computed from the same augmented operands. Output: 3 partial sums per core.

Host: gathers (indexing by matches/sel is part of input sharding), final
8-way sum of the 3 partials. The hardest-negative terms are exactly zero
whenever every row-min distance exceeds NEG_THRESH (true with huge margin for
this problem's data: min distance ~2.9 vs 1.4), in which case the pair-mask
cannot affect the result. If a nonzero negative sum is ever observed the
kernel falls back to an exact host recomputation (mask included).
"""

import numpy as np

import concourse.bacc as bacc
import concourse.mybir as mybir
import concourse.tile as tile
from concourse.bass_utils import run_bass_kernel_spmd

N_CORES = 8
N_PTS = 100000
D = 32
P = 16384
M = 8192
P_LOC = P // N_CORES            # 2048 rows per core
RT = P_LOC // 128               # 16 row tiles
CHUNK = 1024                    # q columns per PSUM tile (2 banks)
NCH = M // CHUNK                # 8 chunks
KA = D + 1                      # contraction dim with augmentation row
POS_THRESH = 0.1
NEG_THRESH = 1.4

F32 = mybir.dt.float32
BF16 = mybir.dt.bfloat16
AX = mybir.AxisListType
ALU = mybir.AluOpType
ACT = mybir.ActivationFunctionType

_CACHED_NC = None
LAST_RESULTS = None            # test.py reads .exec_time_ns after a traced run


def _register_const(nc, value):
    t = nc.alloc_sbuf_tensor(f"const-float32-{value}", [128, 1], F32)
    nc.gpsimd.memset(t.ap(), value)
    nc.const_aps.aps[(F32, value)] = t.ap()


def _register_min2():
    """Custom DVE op: out = min(in0, in1) elementwise, accum_out[p] =
    min(s0, min_k out[p, k]). Consumes TWO streams per cycle (rd0 + rd1),
    doubling reduction throughput vs stock tensor_reduce (which is capped at
    one element/lane/cycle). Registered at runtime into dve_ops.OPS so the
    per-NEFF DVE table generator can resolve it by name."""
    import concourse.dve_ops as dops
    from concourse.dve_spec import C0, Spec, Src0, Src1, _has_src1, lower, minn
    from concourse.dve_uop import DveOpSpec

    name = "MIN2_STREAMS_ANT"
    for op in dops.OPS:
        if op.name == name:
            return op

    def ref(in0, in1, s0, s1, imm2):
        b = np.minimum(in0, in1).astype(np.float32)
        acc = np.minimum(b.reshape(b.shape[0], -1).min(-1, keepdims=True),
                         np.asarray(s0, np.float32).reshape(-1, 1))
        return b, acc

    spec = Spec(body=minn(Src0, Src1), accum=minn, accum_init=C0, reference=ref)
    row = dops._CUSTOM_DVE_ROW_BASE + len(dops.OPS)
    shas = {}
    for ver in ("v3", "v4"):
        uops = lower(spec, ver=ver)
        shas[ver] = DveOpSpec(name=name, opcode=row, uops=uops,
                              rd1_en=_has_src1(spec)).sha(ver)
    op = dops.DveOp(name, spec, subdim=False, uops_sha=shas)
    dops.OPS.append(op)
    dops.CUSTOM_DVE_SPECS[name] = spec
    dops._SUB_OPCODE_FOR_NAME[name] = row
    return op


def _build_nc():
    min2 = _register_min2()
    nc = bacc.Bacc("TRN2", debug=False, target_bir_lowering=False,
                   num_devices=N_CORES)
    for v in (-POS_THRESH, 1e-7, NEG_THRESH):
        _register_const(nc, v)
    nc.all_engine_barrier()
    # fp32 pos-pair operands (positive loss needs full precision);
    # bf16 copies feed the distance-matrix matmuls (fp32 PE matmul streams at
    # 1/4 rate; bf16 error on a distance is ~0.05 vs a 1.5 threshold margin).
    lhsA = nc.dram_tensor("lhsA", [KA, P_LOC], F32, kind="ExternalInput").ap()
    lhsB = nc.dram_tensor("lhsB", [KA, P_LOC], F32, kind="ExternalInput").ap()
    lhsAh = nc.dram_tensor("lhsAh", [KA, P_LOC], BF16, kind="ExternalInput").ap()
    lhsBh = nc.dram_tensor("lhsBh", [KA, P_LOC], BF16, kind="ExternalInput").ap()
    rhsAh = nc.dram_tensor("rhsAh", [KA, M], BF16, kind="ExternalInput").ap()
    rhsBh = nc.dram_tensor("rhsBh", [KA, M], BF16, kind="ExternalInput").ap()
    pnA = nc.dram_tensor("pnA", [128, RT], F32, kind="ExternalInput").ap()
    pnB = nc.dram_tensor("pnB", [128, RT], F32, kind="ExternalInput").ap()
    ones = nc.dram_tensor("ones", [128, 1], F32, kind="ExternalInput").ap()
    outd = nc.dram_tensor("out", [1, 3], F32, kind="ExternalOutput").ap()

    with tile.TileContext(nc) as tc:
        with (
            tc.tile_pool(name="ops", bufs=1) as ops,
            tc.tile_pool(name="wk", bufs=2) as wk,
            tc.tile_pool(name="ps", bufs=4, space="PSUM") as ps,
        ):
            t_lhsA = ops.tile([KA, P_LOC], F32, tag="lhsA")
            t_lhsB = ops.tile([KA, P_LOC], F32, tag="lhsB")
            # bf16 operands are loaded TWICE: rows 0..32 and rows 64..96, so
            # two row-tiles' matmuls can run concurrently on the two 64-row
            # groups of the PE array (K=33 rounds up to a 64-row group).
            t_lhsAh = ops.tile([128, P_LOC], BF16, tag="lhsAh")
            t_lhsBh = ops.tile([128, P_LOC], BF16, tag="lhsBh")
            t_rhsAh = ops.tile([128, M], BF16, tag="rhsAh")
            t_rhsBh = ops.tile([128, M], BF16, tag="rhsBh")
            t_pnA = ops.tile([128, RT], F32, tag="pnA")
            t_pnB = ops.tile([128, RT], F32, tag="pnB")
            t_ones = ops.tile([128, 1], F32, tag="ones")

            # operand loads, critical-path first: the bf16 matmul operands
            # gate the PE pipeline; fp32 pos-path operands are needed only at
            # the tail (the pos path runs after the distance loop).
            for base in (0, 64):
                nc.sync.dma_start(t_lhsAh[base:base + KA, :], lhsAh[:])
            for t_rhs, rhs_d in ((t_rhsAh, rhsAh), (t_rhsBh, rhsBh)):
                for k in range(NCH):
                    sl = slice(k * CHUNK, (k + 1) * CHUNK)
                    for base in (0, 64):
                        nc.sync.dma_start(t_rhs[base:base + KA, sl], rhs_d[:, sl])
                if t_rhs is t_rhsAh:
                    for base in (0, 64):
                        nc.sync.dma_start(t_lhsBh[base:base + KA, :], lhsBh[:])
            nc.sync.dma_start(t_pnA[:], pnA[:])
            nc.sync.dma_start(t_pnB[:], pnB[:])
            nc.sync.dma_start(t_ones[:], ones[:])
            nc.sync.dma_start(t_lhsA[:], lhsA[:])
            nc.sync.dma_start(t_lhsB[:], lhsB[:])

            # ---- the two distance matrices: column-min per row ----
            # Row-tiles are processed in pairs (PE row-groups 0 and 64). Per
            # row-tile, chunk pairs (even PSUM, odd copied to SBUF by ScalarE)
            # feed the 2-stream custom DVE min, halving VectorE time.
            NPAIR = NCH // 2          # 4 min results per row tile
            t_cminA = ops.tile([128, RT * NPAIR], F32, tag="cminA")
            t_cminB = ops.tile([128, RT * NPAIR], F32, tag="cminB")
            for t_lhs, t_rhs, t_cmin in (
                (t_lhsAh, t_rhsAh, t_cminA),
                (t_lhsBh, t_rhsBh, t_cminB),
            ):
                for pr in range(RT // 2):
                    held = {}
                    for k in range(NCH):
                        for half in (0, 1):
                            r = 2 * pr + half
                            base = 64 * half
                            w = t_lhs[base:base + KA, r * 128:(r + 1) * 128]
                            q = ps.tile([128, CHUNK], F32, tag="q")
                            for j in range(CHUNK // 512):
                                c0 = k * CHUNK + j * 512
                                nc.tensor.matmul(
                                    q[:, j * 512:(j + 1) * 512], w,
                                    t_rhs[base:base + KA, c0:c0 + 512])
                            if k % 2 == 0:
                                held[half] = q
                            else:
                                qc = wk.tile([128, CHUNK], F32, tag="qc",
                                             bufs=3)
                                nc.scalar.copy(qc[:], q[:])
                                junk = wk.tile([128, CHUNK], F32, tag="junk",
                                               bufs=2)
                                col = r * NPAIR + k // 2
                                nc.vector._custom_dve(
                                    min2, out=junk[:], in0=held[half][:],
                                    in1=qc[:], s0=3.0e38,
                                    accum_out=t_cmin[:, col:col + 1])

            # ---- positive-pair loss: relu(sum_d (p0-p1)^2 - 0.1), summed ----
            # Runs after the distance loop: its fp32 matmuls (1/4-rate PE)
            # fill the PE tail while VectorE/ScalarE drain the last chunks.
            t_dif = ops.tile([KA, P_LOC], F32, tag="dif")
            nc.vector.tensor_tensor(t_dif[:], t_lhsA[:], t_lhsB[:], ALU.subtract)
            t_difsq = ops.tile([KA, P_LOC], F32, tag="difsq")
            nc.scalar.activation(t_difsq[:], t_dif[:], ACT.Square)
            t_posr = ops.tile([1, P_LOC], F32, tag="posr")
            for j in range(P_LOC // 512):
                pp = ps.tile([1, 512], F32, tag="q")
                nc.tensor.matmul(pp[:], t_ones[0:KA, 0:1],
                                 t_difsq[:, j * 512:(j + 1) * 512])
                nc.scalar.activation(t_posr[0:1, j * 512:(j + 1) * 512], pp[:],
                                     ACT.Relu, bias=-POS_THRESH)

            # ---- epilogue: dist -> relu(1.4 - dist)^2 -> row sums ----
            t_outsb = wk.tile([1, 3], F32, tag="outsb")
            nc.vector.tensor_reduce(out=t_outsb[0:1, 0:1], in_=t_posr[:],
                                    axis=AX.X, op=ALU.add)
            for idx, (t_cmin, t_pn) in enumerate(
                    ((t_cminA, t_pnA), (t_cminB, t_pnB))):
                minq = wk.tile([128, RT], F32, tag="minq")
                nc.vector.tensor_reduce(
                    out=minq[:],
                    in_=t_cmin.rearrange("p (r k) -> p r k", k=NPAIR),
                    axis=AX.X, op=ALU.min)
                d2 = wk.tile([128, RT], F32, tag="d2")
                nc.vector.tensor_tensor(d2[:], minq[:], t_pn[:], ALU.add)
                d2c = wk.tile([128, RT], F32, tag="d2c")
                nc.scalar.activation(d2c[:], d2[:], ACT.Relu)
                dist = wk.tile([128, RT], F32, tag="dist")
                nc.scalar.activation(dist[:], d2c[:], ACT.Sqrt, bias=1e-7)
                y = wk.tile([128, RT], F32, tag="y")
                nc.scalar.activation(y[:], dist[:], ACT.Relu,
                                     bias=NEG_THRESH, scale=-1.0)
                n2 = wk.tile([128, RT], F32, tag="n2")
                nc.scalar.activation(n2[:], y[:], ACT.Square)
                sm = wk.tile([128, 1], F32, tag="sm")
                nc.vector.tensor_reduce(out=sm[:], in_=n2[:], axis=AX.X,
                                        op=ALU.add)
                fp = ps.tile([1, 1], F32, tag="q")
                nc.tensor.matmul(fp[:], sm[:], t_ones[:])
                nc.scalar.copy(t_outsb[0:1, idx + 1:idx + 2], fp[0:1, 0:1])

            nc.sync.dma_start(outd[:], t_outsb[:])

    nc.compile()
    return nc


def _prep_inputs(F0, F1, matches, sel0, sel1):
    posF0 = F0[matches[:, 0]]
    posF1 = F1[matches[:, 1]]
    subF0 = F0[sel0]
    subF1 = F1[sel1]
    import ml_dtypes

    bf16 = ml_dtypes.bfloat16
    ones_col = np.ones((1, P_LOC), np.float32)
    rhsA = np.ascontiguousarray(
        np.concatenate([-2.0 * subF1.T, (subF1 * subF1).sum(1)[None, :]], 0),
        dtype=np.float32)
    rhsB = np.ascontiguousarray(
        np.concatenate([-2.0 * subF0.T, (subF0 * subF0).sum(1)[None, :]], 0),
        dtype=np.float32)
    rhsAh = np.ascontiguousarray(rhsA, dtype=bf16)
    rhsBh = np.ascontiguousarray(rhsB, dtype=bf16)
    ones_in = np.ones((128, 1), np.float32)
    in_maps = []
    for c in range(N_CORES):
        sl = slice(c * P_LOC, (c + 1) * P_LOC)
        p0, p1 = posF0[sl], posF1[sl]
        lhsA = np.ascontiguousarray(
            np.concatenate([p0.T, ones_col], 0), dtype=np.float32)
        lhsB = np.ascontiguousarray(
            np.concatenate([p1.T, ones_col], 0), dtype=np.float32)
        in_maps.append({
            "lhsA": lhsA,
            "lhsB": lhsB,
            "lhsAh": np.ascontiguousarray(lhsA, dtype=bf16),
            "lhsBh": np.ascontiguousarray(lhsB, dtype=bf16),
            "rhsAh": rhsAh,
            "rhsBh": rhsBh,
            "pnA": np.ascontiguousarray(
                (p0 * p0).sum(1).reshape(RT, 128).T, dtype=np.float32),
            "pnB": np.ascontiguousarray(
                (p1 * p1).sum(1).reshape(RT, 128).T, dtype=np.float32),
            "ones": ones_in,
        })
    return in_maps


def _exact_host_reference(F0, F1, matches, sel0, sel1):
    """Bit-faithful numpy port of the oracle, used only as a fallback when a
    nonzero hardest-negative sum is observed (mask handling then matters)."""
    hash_seed = max(F0.shape[0], F1.shape[0])
    pos_ind0 = matches[:, 0].astype(np.int64)
    pos_ind1 = matches[:, 1].astype(np.int64)
    posF0, posF1 = F0[pos_ind0], F1[pos_ind1]
    subF0, subF1 = F0[sel0], F1[sel1]

    def pd(A, B):
        d2 = ((A * A).sum(1)[:, None] + (B * B).sum(1)[None, :]
              - 2.0 * (A @ B.T))
        return np.sqrt(np.maximum(d2, 0.0) + 1e-7)

    D01 = pd(posF0, subF1)
    D10 = pd(posF1, subF0)
    D01min, D10min = D01.min(1), D10.min(1)
    D01ind = np.asarray(sel1)[np.argmin(D01, 1)].astype(np.int64)
    D10ind = np.asarray(sel0)[np.argmin(D10, 1)].astype(np.int64)
    pos_keys = pos_ind0 + pos_ind1 * hash_seed
    mask0 = ~np.isin(pos_ind0 + D01ind * hash_seed, pos_keys)
    mask1 = ~np.isin(D10ind + pos_ind1 * hash_seed, pos_keys)
    pos_loss = np.mean(np.maximum(((posF0 - posF1) ** 2).sum(1) - POS_THRESH, 0))
    n0 = np.maximum(NEG_THRESH - D01min, 0) ** 2
    n1 = np.maximum(NEG_THRESH - D10min, 0) ** 2
    neg0 = (n0 * mask0).sum() / max(mask0.sum(), 1)
    neg1 = (n1 * mask1).sum() / max(mask1.sum(), 1)
    return np.float32(pos_loss + (neg0 + neg1) / 2.0)


def kernel(F0, F1, matches, sel0, sel1):
    global _CACHED_NC, LAST_RESULTS
    F0 = np.ascontiguousarray(np.asarray(F0), dtype=np.float32)
    F1 = np.ascontiguousarray(np.asarray(F1), dtype=np.float32)
    matches = np.asarray(matches)
    sel0 = np.asarray(sel0)
    sel1 = np.asarray(sel1)
    assert F0.shape == (N_PTS, D) and matches.shape == (P, 2)
    assert sel0.shape == (M,) and sel1.shape == (M,)

    in_maps = _prep_inputs(F0, F1, matches, sel0, sel1)
    if _CACHED_NC is None:
        _CACHED_NC = _build_nc()
    try:
        res = run_bass_kernel_spmd(_CACHED_NC, in_maps, list(range(N_CORES)))
    except Exception:
        # a wedged NeuronCore (e.g. NRT_EXEC_UNIT_UNRECOVERABLE from an
        # earlier crashed session) is recoverable via the axon reset call
        try:
            import ctypes

            lib = ctypes.CDLL("/opt/axon/libaxon_pjrt.so")
            lib.axon_reset.restype = ctypes.c_int64
            lib.axon_reset()
        except Exception:
            pass
        res = run_bass_kernel_spmd(_CACHED_NC, in_maps, list(range(N_CORES)))
    LAST_RESULTS = res
    outs = np.stack([r["out"] for r in res.results])   # (8, 1, 3)
    pos_sum = float(outs[:, 0, 0].sum())
    sA = float(outs[:, 0, 1].sum())
    sB = float(outs[:, 0, 2].sum())
    if sA != 0.0 or sB != 0.0:
        # hardest negatives crossed NEG_THRESH: the pair-mask now matters.
        return _exact_host_reference(F0, F1, matches, sel0, sel1)
    return np.float32(pos_sum / P)
